# revision 35
# baseline (speedup 1.0000x reference)
"""Trainium2 Bass kernel for nn_Decoder_22196390985918 (SPADE-style decoder).

Sharding: 8 cores = (batch b in 0..3) x (H-half in 0..1). Each core computes
out[b, :, h0:h0+64, :] for h0 = 64*(core%2).

Key algorithmic transform: the [B, 512, H, W] "middle" tensor (masked scatter
of per-region style vectors mu[b,j,:]) is never materialized. Since
middle[b,:,h,w] = mu[b, j*(h,w), :] with j* the last active region,
conv(middle) collapses to a conv over the 5 one-hot region masks sel_j with
per-batch tap tables G[j, cc, tap] = sum_k Wconv[cc, k, tap] * mu[b, j, k].
That turns ~77 GFLOP of 512-channel convs into one K=45 matmul per tile.

v2 layout: all heavy tensors travel in bf16 (fp32 PSUM accumulation); the
fc Linear (mu) runs on the PE against host-pretransposed fc weights; the
region conv tap weights are host-pretransposed so no device transposes are
needed (the sigmoid gamma/beta blend factors are folded into two scaled
copies of mu^T instead); x is loaded once (the per-core 64-row slice is a
view of the full plane used for the instance-norm stats); the beta-half
partition shift runs as an SBUF->SBUF DMA instead of a PE matmul.
"""
import numpy as np
import ml_dtypes

import concourse.bacc as bacc
import concourse.bass as bass
import concourse.mybir as mybir
import concourse.tile as tile
from concourse.bass_utils import run_bass_kernel_spmd

dt = mybir.dt
F32 = dt.float32
BF16 = dt.bfloat16
AF = mybir.ActivationFunctionType
ALU = mybir.AluOpType
NPBF = ml_dtypes.bfloat16

B, C, H, W, F, L, NH = 4, 64, 128, 128, 5, 512, 128
GW = 130                    # padded grid width  (image col = grid col - 1)
SR = 66                     # seg/sel/actv grid rows (image row = h0 - 1 + r)
MR = 68                     # mask grid rows (image row = h0 - 2 + r)
SEG_N = SR * GW             # 8580
MASK_N = MR * GW            # 8840
SEG_SZ = SEG_N + 2 * GW + 2 + 520   # sel tail slack for im2col windows
MASK_SZ = MASK_N + 2 * GW + 2 + 390
ROWS = 64                   # output rows per core
NCH = 16                    # main conv chunks (4 rows x 128 cols, N=512)
ACH = 22                    # shared conv chunks (3 rows x 128 cols, N=384)
NCORES = 8


def _build_nc():
    nc = bacc.Bacc()

    # ---- per-core DRAM inputs -------------------------------------------
    xb = nc.dram_tensor("xb", [128, H * W // 2], BF16, kind="ExternalInput")
    segg = nc.dram_tensor("segg", [F, SEG_SZ], BF16, kind="ExternalInput")
    maskg = nc.dram_tensor("maskg", [3, MASK_SZ], BF16, kind="ExternalInput")
    codes2 = nc.dram_tensor("codes2", [128, 4 * F * F], BF16,
                            kind="ExternalInput")
    fcwT = nc.dram_tensor("fcwT", [128, F * 4 * 512], BF16, kind="ExternalInput")
    wctd = nc.dram_tensor("wctd", [128, 4 * 9 * 128], BF16, kind="ExternalInput")
    sgbd = nc.dram_tensor("sgbd", [128, NH * 9], BF16, kind="ExternalInput")
    sswT = nc.dram_tensor("sswT", [27, NH], BF16, kind="ExternalInput")
    u5 = nc.dram_tensor("u5", [45, 45], BF16, kind="ExternalInput")
    ident = nc.dram_tensor("ident", [128, 128], BF16, kind="ExternalInput")
    fcb = nc.dram_tensor("fcb", [F, L], BF16, kind="ExternalInput")
    cpkd = nc.dram_tensor("cpkd", [128, 8], F32, kind="ExternalInput")
    out_d = nc.dram_tensor("out", [C, NCH, 512], F32, kind="ExternalOutput")
    import os as _os
    DBG = _os.environ.get("KDBG") == "1"
    if DBG:
        dbg_mur = nc.dram_tensor("dbg_mur", [F, L], BF16, kind="ExternalOutput")
        dbg_selG = nc.dram_tensor("dbg_selG", [45, 128], BF16, kind="ExternalOutput")
        dbg_actv = nc.dram_tensor("dbg_actv", [NH, 3 * GW], BF16, kind="ExternalOutput")
        dbg_spT = nc.dram_tensor("dbg_spT", [128, 2 * 128], BF16, kind="ExternalOutput")
        dbg_sel45 = nc.dram_tensor("dbg_sel45", [45, 512], BF16, kind="ExternalOutput")
        dbg_muTg = nc.dram_tensor("dbg_muTg", [128, 4 * F], BF16, kind="ExternalOutput")
        dbg_stats = nc.dram_tensor("dbg_stats", [C, 2], F32, kind="ExternalOutput")
        dbg_gb = nc.dram_tensor("dbg_gb", [128, 512], BF16, kind="ExternalOutput")
        dbg_wct = nc.dram_tensor("dbg_wct", [128, 9 * 128], BF16, kind="ExternalOutput")
        dbg_gst = nc.dram_tensor("dbg_gst", [F, 9 * 128], BF16, kind="ExternalOutput")

    with tile.TileContext(nc) as tc:
        with (
            tc.tile_pool(name="const", bufs=1) as cst,
            tc.tile_pool(name="gb", bufs=3) as gbp,
            tc.tile_pool(name="pbt", bufs=3) as pbtp,
            tc.tile_pool(name="xn", bufs=3) as xnp,
            tc.tile_pool(name="pmain", bufs=2, space="PSUM") as pmain,
            tc.tile_pool(name="paux", bufs=2, space="PSUM") as paux,
            tc.tile_pool(name="gpsp", bufs=3, space="PSUM") as gpsp,
            tc.tile_pool(name="pmu", bufs=1, space="PSUM") as pmu,
        ):
            # ---- grids + fcwT on sync, nothing tiny ahead of them -------
            sel45 = cst.tile([45, SEG_N], BF16)
            segp = segg[:].ap[0][0]
            for ty in range(3):
                srcg = bass.AP(tensor=segg[:].tensor, offset=ty * GW,
                               ap=[[1, 3], [segp, F], [1, SEG_N]])
                nc.sync.dma_start(out=sel45[15 * ty:15 * ty + 15, :], in_=srcg)
            mask27 = cst.tile([27, MASK_N], BF16)
            maskp_ = maskg[:].ap[0][0]
            for ty in range(3):
                srcg = bass.AP(tensor=maskg[:].tensor, offset=ty * GW,
                               ap=[[1, 3], [maskp_, 3], [1, MASK_N]])
                nc.sync.dma_start(out=mask27[9 * ty:9 * ty + 9, :], in_=srcg)
            fw_sb = cst.tile([128, F, 4, 512], BF16)
            for j in range(F):
                nc.sync.dma_start(
                    out=fw_sb[:, j, :, :].rearrange("p a k -> p (a k)"),
                    in_=fcwT[:, j * 2048:(j + 1) * 2048])
            xb_sb = cst.tile([128, H * W // 2], BF16)
            # gpsimd queue: packed consts + memsets, then weights
            cpk = cst.tile([128, 8], F32)
            nc.gpsimd.dma_start(out=cpk[:], in_=cpkd[:])
            convb = cpk[:, 0:1]
            spadeb = cpk[:, 1:2]
            ssb_t = cpk[0:NH, 2:3]
            graw = cpk[:, 5:6]
            braw = cpk[:, 6:7]
            id_bf = cst.tile([128, 128], BF16)
            nc.gpsimd.dma_start(out=id_bf[:], in_=ident[:])
            cT = cst.tile([128, 4, F, F], BF16)
            nc.gpsimd.dma_start(out=cT[:].rearrange("p a j k -> p (a j k)"),
                                in_=codes2[:])
            fcb_sb = cst.tile([F, L], BF16)
            nc.gpsimd.dma_start(out=fcb_sb[:], in_=fcb[:])
            ones_t = cst.tile([128, 1], F32)
            nc.gpsimd.memset(ones_t[:], 1.0)
            eps_t = cst.tile([C, 1], F32)
            nc.gpsimd.memset(eps_t[:], 1e-5)
            half1 = cst.tile([128, 1], F32)
            nc.gpsimd.memset(half1[0:64, :], 1.0)
            nc.gpsimd.memset(half1[64:128, :], 0.0)
            zsb = cst.tile([128, 132], BF16)
            nc.gpsimd.memset(zsb[:], 0.0)
            u5r = cst.tile([45, 45], BF16)
            nc.gpsimd.dma_start(out=u5r[:], in_=u5[:])
            sswT_sb = cst.tile([27, NH], BF16)
            nc.gpsimd.dma_start(out=sswT_sb[:], in_=sswT[:])
            sgb = cst.tile([128, NH * 9], BF16)
            nc.gpsimd.dma_start(out=sgb[:], in_=sgbd[:])
            wct_sb = cst.tile([128, 4, 9, 128], BF16)
            nc.gpsimd.dma_start(
                out=wct_sb[:].rearrange("p a t c -> p (a t c)"), in_=wctd[:])

            # blending factors (scalar queue, tiny)
            gsig = cst.tile([128, 1], F32)
            nc.scalar.activation(gsig[:], graw, AF.Sigmoid)
            bsig = cst.tile([128, 1], F32)
            nc.scalar.activation(bsig[:], braw, AF.Sigmoid)
            gba = cst.tile([128, 1], F32)
            nc.vector.tensor_copy(gba[0:64, :], gsig[0:64, :])
            nc.vector.tensor_copy(gba[64:128, :], bsig[64:128, :])
            om_gba = cst.tile([128, 1], F32)
            nc.scalar.activation(om_gba[:], gba[:], AF.Identity,
                                 bias=ones_t[:], scale=-1.0)
            tb1 = cst.tile([128, 1], F32)
            nc.vector.tensor_mul(tb1[:], convb, gba[:])
            tb2 = cst.tile([128, 1], F32)
            nc.vector.tensor_mul(tb2[:], spadeb, om_gba[:])
            bias_t = cst.tile([128, 1], F32)
            nc.vector.tensor_add(bias_t[:], tb1[:], tb2[:])
            bias1_t = cst.tile([128, 1], F32)
            nc.vector.tensor_add(bias1_t[:], bias_t[:], half1[:])
            ones128 = cst.tile([128, 128], F32)
            nc.gpsimd.memset(ones128[:], 1.0)
            blendT = cst.tile([128, 128], BF16)
            nc.scalar.activation(blendT[:, 0:64], ones128[:, 0:64], AF.Copy,
                                 scale=gsig[:])
            nc.scalar.activation(blendT[:, 64:128], ones128[:, 64:128],
                                 AF.Copy, scale=bsig[:])
            for q in range(4):
                nc.scalar.dma_start(out=xb_sb[:, q * 2048:(q + 1) * 2048],
                                    in_=xb[:, q * 2048:(q + 1) * 2048])

            # ---- PE: open the mu accumulation group with the fc bias ----
            mu_ps = pmu.tile([F, L], F32, tag="mu", name="mu_ps")
            nc.tensor.matmul(mu_ps[:], id_bf[0:F, 0:F], fcb_sb[:],
                             start=True, stop=False)

            # ---- PE: region count matmuls (grids land first) ------------
            t_sb = cst.tile([45, SEG_N], BF16)
            segchunks = []
            off = 0
            while off < SEG_N:
                n = min(512, SEG_N - off)
                segchunks.append((off, n))
                off += n
            for off, n in segchunks:
                pc = paux.tile([45, 512], F32, tag="aux")
                nc.tensor.matmul(pc[:, 0:n], u5r[:], sel45[:, off:off + n],
                                 start=True, stop=True)
                nc.vector.tensor_scalar(t_sb[:, off:off + n], pc[:, 0:n],
                                        0.0, None, op0=ALU.is_equal)

            # ---- PE: shared conv (mask 3 -> NH), pre-shifted rows -------
            actv = cst.tile([NH, SR, GW], BF16)
            bord = actv[:, :, 0:1]
            nc.vector.tensor_copy(
                bass.AP(tensor=bord.tensor, offset=bord.offset,
                        ap=[bord.ap[0], [GW, SR], [GW - 1, 2]]),
                zsb[:].rearrange("p (a b) -> p a b", a=SR))
            m3 = mask27[:].rearrange("p (r c) -> p r c", c=GW)
            for a in range(ACH):
                r = 3 * a
                psh = paux.tile([NH, 3, 128], F32, tag="aux")
                nc.tensor.matmul(psh[:], sswT_sb[:], m3[:, r:r + 3, 0:128],
                                 start=True, stop=True)
                nc.scalar.activation(actv[:, r:r + 3, 1:129], psh[:], AF.Relu,
                                     bias=ssb_t, scale=1.0)

            # ---- PE: mu matmuls (fc Linear, all regions, one group) -----
            for j in range(F):
                for lc in range(4):
                    nc.tensor.matmul(mu_ps[:], cT[:, lc, j, :],
                                     fw_sb[:, j, lc, :],
                                     start=False, stop=(j == F - 1 and lc == 3))

            # ---- mu epilogue: relu, transpose ---------------------------
            mur = cst.tile([F, L], BF16)
            nc.scalar.activation(mur[:], mu_ps[:], AF.Relu)
            muT = cst.tile([128, 4, F], BF16)
            for kb in range(4):
                mt = paux.tile([128, F], BF16, tag="aux")
                nc.tensor.transpose(mt[:], mur[:, kb * 128:(kb + 1) * 128],
                                    id_bf[0:F, 0:F])
                nc.scalar.activation(muT[:, kb, :], mt[:], AF.Copy)

            # ---- PE: G matmuls, selG assembly spread over 3 queues ------
            gps = [gpsp.tile([F, 3, 128], F32, tag="gps", name=f"gps{_g}")
                   for _g in range(3)]
            for kb in range(4):
                for g in range(3):
                    nc.tensor.matmul(gps[g][:], muT[:, kb, :],
                                     wct_sb[:, kb, 3 * g:3 * g + 3, :],
                                     start=(kb == 0), stop=(kb == 3))
            selG = cst.tile([45, 128], BF16)
            gstage = cst.tile([F, 9, 128], BF16)
            for g in range(3):
                nc.scalar.activation(gstage[:, 3 * g:3 * g + 3, :],
                                     gps[g][:], AF.Copy)
            qs = [nc.sync, nc.gpsimd, nc.scalar]
            for t in range(9):
                qs[t % 3].dma_start(out=selG[F * t:F * t + F, :],
                                    in_=gstage[:, t, :])

            # ---- spade gamma/beta lhsT: DVE scale + 9 PE transposes -----
            nc.vector.tensor_scalar_mul(sgb[:], sgb[:], om_gba[:])
            spT = cst.tile([128, 9, 128], BF16)
            sgb3 = sgb[:].rearrange("p (l t) -> p l t", t=9)
            for t in range(9):
                pt = gpsp.tile([128, 128], BF16, tag="gps")
                nc.tensor.transpose(pt[:], sgb3[:, :, t], id_bf[:])
                nc.scalar.activation(spT[:, t, :], pt[:], AF.Copy)

            # ---- DVE: sel = seg * t, blend, then instance-norm stats ----
            stats_t = cst.tile([128, 16, 6], F32)
            x16 = xb_sb[:].rearrange("c (k n) -> c k n", k=16)
            for off, n in segchunks:
                nc.vector.tensor_mul(sel45[:, off:off + n],
                                     sel45[:, off:off + n],
                                     t_sb[:, off:off + n])
            nc.vector.tensor_scalar_mul(actv[:, 0, :], actv[:, 0, :],
                                        cpk[:, 3:4])
            nc.vector.tensor_scalar_mul(actv[:, SR - 1, :], actv[:, SR - 1, :],
                                        cpk[:, 4:5])
            for k in range(8):
                nc.vector.bn_stats(out=stats_t[:, k, :], in_=x16[:, k, :])
            # blend gamma/beta halves of selG while gstage lands
            nc.vector.tensor_mul(selG[:], selG[:], blendT[0:45, :])
            for k in range(8, 16):
                nc.vector.bn_stats(out=stats_t[:, k, :], in_=x16[:, k, :])

            # ---- instance-norm stats merge (packed half planes) ---------
            mv = cst.tile([128, 2], F32)
            nc.vector.bn_aggr(out=mv[:], in_=stats_t[:])
            mvb = cst.tile([C, 2], F32)
            nc.sync.dma_start(out=mvb[:], in_=mv[64:128, :])
            msum = cst.tile([C, 1], F32)
            nc.vector.tensor_add(msum[:], mv[0:64, 0:1], mvb[:, 0:1])
            mdif = cst.tile([C, 1], F32)
            nc.vector.tensor_sub(mdif[:], mv[0:64, 0:1], mvb[:, 0:1])
            vsum = cst.tile([C, 1], F32)
            nc.vector.tensor_add(vsum[:], mv[0:64, 1:2], mvb[:, 1:2])
            dm2 = cst.tile([C, 1], F32)
            nc.vector.tensor_mul(dm2[:], mdif[:], mdif[:])
            var_t = cst.tile([C, 1], F32)
            nc.vector.tensor_scalar(var_t[:], dm2[:], 0.25, None, op0=ALU.mult)
            nc.vector.tensor_scalar(vsum[:], vsum[:], 0.5, None, op0=ALU.mult)
            nc.vector.tensor_add(var_t[:], var_t[:], vsum[:])
            sd = cst.tile([C, 1], F32)
            nc.scalar.activation(sd[:], var_t[:], AF.Sqrt, bias=eps_t[:],
                                 scale=1.0)
            rstd = cst.tile([C, 1], F32)
            nc.vector.reciprocal(rstd[:], sd[:])
            nbias = cst.tile([C, 1], F32)
            nc.vector.tensor_scalar(nbias[:], msum[:], 0.5, None, op0=ALU.mult)
            nc.vector.tensor_mul(nbias[:], nbias[:], rstd[:])
            nc.vector.tensor_scalar_mul(nbias[:], nbias[:], -1.0)

            # ---- main conv + epilogue (epilogue one chunk behind) -------
            s3 = sel45[:].rearrange("p (r c) -> p r c", c=GW)
            pms = {}

            def conv_chunk(i):
                pm = pmain.tile([128, 4, 128], F32, tag="pm", name=f"pm_{i}")
                pms[i] = pm
                for t in range(9):
                    ty, tx = divmod(t, 3)
                    nc.tensor.matmul(pm[:], spT[:, t, :],
                                     actv[:, 4 * i + ty:4 * i + ty + 4,
                                          tx:tx + 128],
                                     start=(t == 0), stop=False)
                nc.tensor.matmul(pm[:], selG[:], s3[:, 4 * i:4 * i + 4, 0:128],
                                 start=False, stop=True)

            def epi_chunk(i):
                pm = pms.pop(i)
                gb = gbp.tile([128, 4, 128], BF16, tag="gb", name=f"gb_{i}")
                nc.scalar.activation(gb[:], pm[:], AF.Identity,
                                     bias=bias1_t[:], scale=1.0)
                pb = pbtp.tile([64, 4, 128], BF16, tag="pbt", name=f"pb_{i}")
                eng = nc.scalar if i % 2 == 0 else nc.gpsimd
                eng.dma_start(out=pb[:], in_=gb[64:128, :, :])
                xsl = xb_sb[0:64, i * 512:(i + 1) * 512]
                xnt = xnp.tile([C, 4, 128], F32, tag="xn", name=f"xnt_{i}")
                nc.gpsimd.tensor_scalar(
                    xnt[:].rearrange("p t c -> p (t c)"), xsl,
                    rstd[:], nbias[:], op0=ALU.mult, op1=ALU.add)
                nc.vector.tensor_mul(xnt[:], xnt[:], gb[0:64, :, :])
                nc.vector.tensor_add(xnt[:].rearrange("p t c -> p (t c)"),
                                     xnt[:].rearrange("p t c -> p (t c)"),
                                     pb[:].rearrange("p t c -> p (t c)"))
                if DBG and i == 3:
                    nc.sync.dma_start(out=dbg_gb[:],
                                      in_=gb[:].rearrange("c r w -> c (r w)"))
                nc.sync.dma_start(out=out_d[:, i, :],
                                  in_=xnt[:].rearrange("c r w -> c (r w)"))

            gb_keep = {}
            conv_chunk(0)
            for i in range(1, NCH):
                conv_chunk(i)
                epi_chunk(i - 1)
            epi_chunk(NCH - 1)
            if DBG:
                nc.sync.dma_start(out=dbg_mur[:], in_=mur[:])
                nc.sync.dma_start(out=dbg_selG[:], in_=selG[:])
                nc.sync.dma_start(out=dbg_actv[:],
                                  in_=actv[:, 1:4, :].rearrange("p a b -> p (a b)"))
                nc.sync.dma_start(out=dbg_spT[:],
                                  in_=spT[:, 0:2, :].rearrange("p a b -> p (a b)"))
                nc.sync.dma_start(out=dbg_sel45[:], in_=sel45[:, 2048:2560])
                nc.sync.dma_start(out=dbg_muTg[:],
                                  in_=muT[:].rearrange("p a b -> p (a b)"))
                nc.sync.dma_start(out=dbg_stats[:, 0:1], in_=rstd[:])
                nc.sync.dma_start(out=dbg_stats[:, 1:2], in_=nbias[:])
                nc.sync.dma_start(out=dbg_wct[:],
                                  in_=wct_sb[:, 0, :, :].rearrange("p a b -> p (a b)"))
                nc.sync.dma_start(out=dbg_gst[:],
                                  in_=gstage[:].rearrange("p a b -> p (a b)"))

    nc.finalize()
    return nc


_NC = None


def _cpk(cgb, cbb, sgbb, sbbb, ssb, bg, bb, h0):
    cpk = np.zeros((128, 8), np.float32)
    cpk[0:64, 0] = cgb; cpk[64:128, 0] = cbb
    cpk[0:64, 1] = sgbb; cpk[64:128, 1] = sbbb
    cpk[:, 2] = ssb
    cpk[:, 3] = 0.0 if h0 == 0 else 1.0
    cpk[:, 4] = 0.0 if h0 + ROWS == H else 1.0
    cpk[:, 5] = bg[0]
    cpk[:, 6] = bb[0]
    return cpk


def kernel(**inputs):
    global _NC
    x = np.asarray(inputs["x"], dtype=np.float32)
    segmap = np.asarray(inputs["segmap"], dtype=np.float32)
    codes_vector = np.asarray(inputs["codes_vector"], dtype=np.float32)
    mask = np.asarray(inputs["mask"], dtype=np.float32)
    fc_w = np.asarray(inputs["fc_w"], dtype=np.float32)
    fc_b = np.asarray(inputs["fc_b"], dtype=np.float32)
    conv_gamma_w = np.asarray(inputs["conv_gamma_w"], dtype=np.float32)
    conv_gamma_b = np.asarray(inputs["conv_gamma_b"], dtype=np.float32)
    conv_beta_w = np.asarray(inputs["conv_beta_w"], dtype=np.float32)
    conv_beta_b = np.asarray(inputs["conv_beta_b"], dtype=np.float32)
    spade_shared_w = np.asarray(inputs["spade_shared_w"], dtype=np.float32)
    spade_shared_b = np.asarray(inputs["spade_shared_b"], dtype=np.float32)
    spade_gamma_w = np.asarray(inputs["spade_gamma_w"], dtype=np.float32)
    spade_gamma_b = np.asarray(inputs["spade_gamma_b"], dtype=np.float32)
    spade_beta_w = np.asarray(inputs["spade_beta_w"], dtype=np.float32)
    spade_beta_b = np.asarray(inputs["spade_beta_b"], dtype=np.float32)
    blending_gamma = np.asarray(inputs["blending_gamma"], dtype=np.float32)
    blending_beta = np.asarray(inputs["blending_beta"], dtype=np.float32)

    if _NC is None:
        _NC = _build_nc()

    # fc weights: mu[j,k] = sum_l codes[j,l] * fc_w[j,k,l] -> rhs tiles
    # [128(l_part), 512(k)] per (j, lc): fcwT[j][p, lc*512+k] = fc_w[j,k,lc*128+p]
    fcwT_h = np.ascontiguousarray(
        fc_w.transpose(0, 2, 1).reshape(F, 4, 128, 512).transpose(2, 0, 1, 3)
        .reshape(128, F * 4 * 512)).astype(NPBF)
    # region conv taps: wct[p, lc, t, c] = Wconv[c, lc*128+p, t]
    wc = np.concatenate([conv_gamma_w, conv_beta_w], axis=0)  # [128c, 512, 3, 3]
    wctd_h = np.ascontiguousarray(
        wc.reshape(128, 512, 9).transpose(1, 2, 0)      # [512l, 9t, 128c]
        .reshape(4, 128, 9, 128).transpose(1, 0, 2, 3)  # [128p, 4lc, 9, 128]
        .reshape(128, 4 * 9 * 128)).astype(NPBF)
    # spade gamma/beta stacked, natural layout (device transposes these 9)
    sgbd_h = np.concatenate(
        [spade_gamma_w.reshape(C, NH * 9), spade_beta_w.reshape(C, NH * 9)],
        axis=0).astype(NPBF)
    # shared conv lhsT [27(ty,tx,ic), NH]
    sswT_h = np.ascontiguousarray(
        spade_shared_w.transpose(2, 3, 1, 0).reshape(27, NH)).astype(NPBF)

    shared = {
        "fcwT": fcwT_h,
        "wctd": wctd_h,
        "sgbd": sgbd_h,
        "sswT": sswT_h,
        "fcb": np.ascontiguousarray(fc_b).astype(NPBF),
        "u5": np.kron(np.eye(9, dtype=np.float32),
                      np.tril(np.ones((F, F), np.float32), -1)).astype(NPBF),
        "ident": np.eye(128, dtype=np.float32).astype(NPBF),
    }

    in_maps = []
    for c in range(NCORES):
        b, half = divmod(c, 2)
        h0 = half * ROWS
        segp = np.zeros((F, SEG_SZ), np.float32)
        segp2 = np.zeros((F, SR, GW), np.float32)
        r_lo, r_hi = h0 - 1, h0 + ROWS + 1  # exclusive
        s_lo, s_hi = max(r_lo, 0), min(r_hi, H)
        segp2[:, s_lo - r_lo:s_hi - r_lo, 1:129] = segmap[b, :, s_lo:s_hi, :]
        segp[:, 0:SR * GW] = segp2.reshape(F, -1)
        maskp = np.zeros((3, MASK_SZ), np.float32)
        maskp2 = np.zeros((3, MR, GW), np.float32)
        m_lo, m_hi = h0 - 2, h0 + ROWS + 2
        ms_lo, ms_hi = max(m_lo, 0), min(m_hi, H)
        maskp2[:, ms_lo - m_lo:ms_hi - m_lo, 1:129] = mask[b, :, ms_lo:ms_hi, :]
        maskp[:, 0:MR * GW] = maskp2.reshape(3, -1)
        # x plane rotated so the core's own 64 rows come first: the epilogue
        # reads columns [i*512, (i+1)*512) directly; stats are rotation-
        # invariant.
        xrot = np.roll(x[b].reshape(C, H, W), -h0, axis=1).reshape(C, H * W)
        xpk = np.concatenate([xrot[:, :H * W // 2], xrot[:, H * W // 2:]], 0)
        # block-diagonal codes lhsT [128(p), 4(lc), F(j), F(col)]: column j of
        # the (lc, j) slice holds codes_vector[b, j, lc*128+p], rest zero.
        cT_full = codes_vector[b].T.reshape(4, 128, F).transpose(1, 0, 2)
        codes5 = np.zeros((128, 4, F, F), np.float32)
        for j in range(F):
            codes5[:, :, j, j] = cT_full[:, :, j]
        codes2_h = np.ascontiguousarray(
            codes5.reshape(128, 4 * F * F)).astype(NPBF)
        in_maps.append(dict(
            shared,
            xb=xpk.astype(NPBF),
            cpkd=_cpk(conv_gamma_b, conv_beta_b, spade_gamma_b, spade_beta_b,
                      spade_shared_b, blending_gamma, blending_beta, h0),
            segg=np.ascontiguousarray(segp).astype(NPBF),
            maskg=np.ascontiguousarray(maskp).astype(NPBF),
            codes2=codes2_h,
        ))

    res = run_bass_kernel_spmd(_NC, in_maps, list(range(NCORES)))

    out = np.empty((B, C, H, W), np.float32)
    for c in range(NCORES):
        b, half = divmod(c, 2)
        h0 = half * ROWS
        out[b, :, h0:h0 + ROWS, :] = res.results[c]["out"].reshape(C, ROWS, W)
    return out


# revision 36
# speedup vs baseline: 1.0012x; 1.0012x over previous
"""Trainium2 Bass kernel for nn_Decoder_22196390985918 (SPADE-style decoder).

Sharding: 8 cores = (batch b in 0..3) x (H-half in 0..1). Each core computes
out[b, :, h0:h0+64, :] for h0 = 64*(core%2).

Key algorithmic transform: the [B, 512, H, W] "middle" tensor (masked scatter
of per-region style vectors mu[b,j,:]) is never materialized. Since
middle[b,:,h,w] = mu[b, j*(h,w), :] with j* the last active region,
conv(middle) collapses to a conv over the 5 one-hot region masks sel_j with
per-batch tap tables G[j, cc, tap] = sum_k Wconv[cc, k, tap] * mu[b, j, k].
That turns ~77 GFLOP of 512-channel convs into one K=45 matmul per tile.

v2 layout: all heavy tensors travel in bf16 (fp32 PSUM accumulation); the
fc Linear (mu) runs on the PE against host-pretransposed fc weights; the
region conv tap weights are host-pretransposed so no device transposes are
needed (the sigmoid gamma/beta blend factors are folded into two scaled
copies of mu^T instead); x is loaded once (the per-core 64-row slice is a
view of the full plane used for the instance-norm stats); the beta-half
partition shift runs as an SBUF->SBUF DMA instead of a PE matmul.
"""
import numpy as np
import ml_dtypes

import concourse.bacc as bacc
import concourse.bass as bass
import concourse.mybir as mybir
import concourse.tile as tile
from concourse.bass_utils import run_bass_kernel_spmd

dt = mybir.dt
F32 = dt.float32
BF16 = dt.bfloat16
AF = mybir.ActivationFunctionType
ALU = mybir.AluOpType
NPBF = ml_dtypes.bfloat16

B, C, H, W, F, L, NH = 4, 64, 128, 128, 5, 512, 128
GW = 130                    # padded grid width  (image col = grid col - 1)
SR = 66                     # seg/sel/actv grid rows (image row = h0 - 1 + r)
MR = 68                     # mask grid rows (image row = h0 - 2 + r)
SEG_N = SR * GW             # 8580
MASK_N = MR * GW            # 8840
SEG_SZ = SEG_N + 2 * GW + 2 + 520   # sel tail slack for im2col windows
MASK_SZ = MASK_N + 2 * GW + 2 + 390
ROWS = 64                   # output rows per core
NCH = 16                    # main conv chunks (4 rows x 128 cols, N=512)
ACH = 22                    # shared conv chunks (3 rows x 128 cols, N=384)
NCORES = 8


def _build_nc():
    nc = bacc.Bacc()

    # ---- per-core DRAM inputs -------------------------------------------
    xb = nc.dram_tensor("xb", [128, H * W // 2], BF16, kind="ExternalInput")
    segg = nc.dram_tensor("segg", [F, SEG_SZ], BF16, kind="ExternalInput")
    maskg = nc.dram_tensor("maskg", [3, MASK_SZ], BF16, kind="ExternalInput")
    codes2 = nc.dram_tensor("codes2", [128, 4 * F * F], BF16,
                            kind="ExternalInput")
    fcwT = nc.dram_tensor("fcwT", [128, F * 4 * 512], BF16, kind="ExternalInput")
    wctd = nc.dram_tensor("wctd", [128, 4 * 9 * 128], BF16, kind="ExternalInput")
    sgbd = nc.dram_tensor("sgbd", [128, NH * 9], BF16, kind="ExternalInput")
    sswT = nc.dram_tensor("sswT", [27, NH], BF16, kind="ExternalInput")
    u5 = nc.dram_tensor("u5", [45, 45], BF16, kind="ExternalInput")
    ident = nc.dram_tensor("ident", [128, 128], BF16, kind="ExternalInput")
    fcb = nc.dram_tensor("fcb", [F, L], BF16, kind="ExternalInput")
    cpkd = nc.dram_tensor("cpkd", [128, 8], F32, kind="ExternalInput")
    out_d = nc.dram_tensor("out", [C, NCH, 512], F32, kind="ExternalOutput")
    import os as _os
    DBG = _os.environ.get("KDBG") == "1"
    if DBG:
        dbg_mur = nc.dram_tensor("dbg_mur", [F, L], BF16, kind="ExternalOutput")
        dbg_selG = nc.dram_tensor("dbg_selG", [45, 128], BF16, kind="ExternalOutput")
        dbg_actv = nc.dram_tensor("dbg_actv", [NH, 3 * GW], BF16, kind="ExternalOutput")
        dbg_spT = nc.dram_tensor("dbg_spT", [128, 2 * 128], BF16, kind="ExternalOutput")
        dbg_sel45 = nc.dram_tensor("dbg_sel45", [45, 512], BF16, kind="ExternalOutput")
        dbg_muTg = nc.dram_tensor("dbg_muTg", [128, 4 * F], BF16, kind="ExternalOutput")
        dbg_stats = nc.dram_tensor("dbg_stats", [C, 2], F32, kind="ExternalOutput")
        dbg_gb = nc.dram_tensor("dbg_gb", [128, 512], BF16, kind="ExternalOutput")
        dbg_wct = nc.dram_tensor("dbg_wct", [128, 9 * 128], BF16, kind="ExternalOutput")
        dbg_gst = nc.dram_tensor("dbg_gst", [F, 9 * 128], BF16, kind="ExternalOutput")

    with tile.TileContext(nc) as tc:
        with (
            tc.tile_pool(name="const", bufs=1) as cst,
            tc.tile_pool(name="gb", bufs=3) as gbp,
            tc.tile_pool(name="pbt", bufs=3) as pbtp,
            tc.tile_pool(name="xn", bufs=3) as xnp,
            tc.tile_pool(name="pmain", bufs=2, space="PSUM") as pmain,
            tc.tile_pool(name="paux", bufs=2, space="PSUM") as paux,
            tc.tile_pool(name="gpsp", bufs=3, space="PSUM") as gpsp,
            tc.tile_pool(name="pmu", bufs=1, space="PSUM") as pmu,
        ):
            # ---- PE-critical loads on the fast scalar ring --------------
            sel45 = cst.tile([45, SEG_N], BF16)
            segp = segg[:].ap[0][0]
            for ty in range(3):
                srcg = bass.AP(tensor=segg[:].tensor, offset=ty * GW,
                               ap=[[1, 3], [segp, F], [1, SEG_N]])
                nc.scalar.dma_start(out=sel45[15 * ty:15 * ty + 15, :],
                                    in_=srcg)
            fw_sb = cst.tile([128, F, 4, 512], BF16)
            for j in range(F):
                nc.scalar.dma_start(
                    out=fw_sb[:, j, :, :].rearrange("p a k -> p (a k)"),
                    in_=fcwT[:, j * 2048:(j + 1) * 2048])
            # x plane on sync (needed latest of the big loads)
            xb_sb = cst.tile([128, H * W // 2], BF16)
            for q in range(4):
                nc.sync.dma_start(out=xb_sb[:, q * 2048:(q + 1) * 2048],
                                  in_=xb[:, q * 2048:(q + 1) * 2048])
            # gpsimd queue: packed consts + memsets, then weights
            cpk = cst.tile([128, 8], F32)
            nc.gpsimd.dma_start(out=cpk[:], in_=cpkd[:])
            convb = cpk[:, 0:1]
            spadeb = cpk[:, 1:2]
            ssb_t = cpk[0:NH, 2:3]
            graw = cpk[:, 5:6]
            braw = cpk[:, 6:7]
            id_bf = cst.tile([128, 128], BF16)
            nc.gpsimd.dma_start(out=id_bf[:], in_=ident[:])
            cT = cst.tile([128, 4, F, F], BF16)
            nc.gpsimd.dma_start(out=cT[:].rearrange("p a j k -> p (a j k)"),
                                in_=codes2[:])
            fcb_sb = cst.tile([F, L], BF16)
            nc.gpsimd.dma_start(out=fcb_sb[:], in_=fcb[:])
            mask27 = cst.tile([27, MASK_N], BF16)
            maskp_ = maskg[:].ap[0][0]
            for ty in range(3):
                srcg = bass.AP(tensor=maskg[:].tensor, offset=ty * GW,
                               ap=[[1, 3], [maskp_, 3], [1, MASK_N]])
                nc.gpsimd.dma_start(out=mask27[9 * ty:9 * ty + 9, :],
                                    in_=srcg)
            ones_t = cst.tile([128, 1], F32)
            nc.gpsimd.memset(ones_t[:], 1.0)
            eps_t = cst.tile([C, 1], F32)
            nc.gpsimd.memset(eps_t[:], 1e-5)
            half1 = cst.tile([128, 1], F32)
            nc.gpsimd.memset(half1[0:64, :], 1.0)
            nc.gpsimd.memset(half1[64:128, :], 0.0)
            zsb = cst.tile([128, 132], BF16)
            nc.gpsimd.memset(zsb[:], 0.0)
            u5r = cst.tile([45, 45], BF16)
            nc.gpsimd.dma_start(out=u5r[:], in_=u5[:])
            sswT_sb = cst.tile([27, NH], BF16)
            nc.gpsimd.dma_start(out=sswT_sb[:], in_=sswT[:])
            sgb = cst.tile([128, NH * 9], BF16)
            nc.gpsimd.dma_start(out=sgb[:], in_=sgbd[:])
            wct_sb = cst.tile([128, 4, 9, 128], BF16)
            nc.gpsimd.dma_start(
                out=wct_sb[:].rearrange("p a t c -> p (a t c)"), in_=wctd[:])

            # blending factors (scalar queue, tiny)
            gsig = cst.tile([128, 1], F32)
            nc.scalar.activation(gsig[:], graw, AF.Sigmoid)
            bsig = cst.tile([128, 1], F32)
            nc.scalar.activation(bsig[:], braw, AF.Sigmoid)
            gba = cst.tile([128, 1], F32)
            nc.vector.tensor_copy(gba[0:64, :], gsig[0:64, :])
            nc.vector.tensor_copy(gba[64:128, :], bsig[64:128, :])
            om_gba = cst.tile([128, 1], F32)
            nc.scalar.activation(om_gba[:], gba[:], AF.Identity,
                                 bias=ones_t[:], scale=-1.0)
            tb1 = cst.tile([128, 1], F32)
            nc.vector.tensor_mul(tb1[:], convb, gba[:])
            tb2 = cst.tile([128, 1], F32)
            nc.vector.tensor_mul(tb2[:], spadeb, om_gba[:])
            bias_t = cst.tile([128, 1], F32)
            nc.vector.tensor_add(bias_t[:], tb1[:], tb2[:])
            bias1_t = cst.tile([128, 1], F32)
            nc.vector.tensor_add(bias1_t[:], bias_t[:], half1[:])
            ones128 = cst.tile([128, 128], F32)
            nc.gpsimd.memset(ones128[:], 1.0)
            blendT = cst.tile([128, 128], BF16)
            nc.scalar.activation(blendT[:, 0:64], ones128[:, 0:64], AF.Copy,
                                 scale=gsig[:])
            nc.scalar.activation(blendT[:, 64:128], ones128[:, 64:128],
                                 AF.Copy, scale=bsig[:])

            # ---- PE: open the mu accumulation group with the fc bias ----
            mu_ps = pmu.tile([F, L], F32, tag="mu", name="mu_ps")
            nc.tensor.matmul(mu_ps[:], id_bf[0:F, 0:F], fcb_sb[:],
                             start=True, stop=False)

            # ---- PE: region count matmuls (grids land first) ------------
            t_sb = cst.tile([45, SEG_N], BF16)
            segchunks = []
            off = 0
            while off < SEG_N:
                n = min(512, SEG_N - off)
                segchunks.append((off, n))
                off += n
            for off, n in segchunks:
                pc = paux.tile([45, 512], F32, tag="aux")
                nc.tensor.matmul(pc[:, 0:n], u5r[:], sel45[:, off:off + n],
                                 start=True, stop=True)
                nc.vector.tensor_scalar(t_sb[:, off:off + n], pc[:, 0:n],
                                        0.0, None, op0=ALU.is_equal)

            # ---- PE: shared conv (mask 3 -> NH), pre-shifted rows -------
            actv = cst.tile([NH, SR, GW], BF16)
            bord = actv[:, :, 0:1]
            nc.vector.tensor_copy(
                bass.AP(tensor=bord.tensor, offset=bord.offset,
                        ap=[bord.ap[0], [GW, SR], [GW - 1, 2]]),
                zsb[:].rearrange("p (a b) -> p a b", a=SR))
            m3 = mask27[:].rearrange("p (r c) -> p r c", c=GW)
            for a in range(ACH):
                r = 3 * a
                psh = paux.tile([NH, 3, 128], F32, tag="aux")
                nc.tensor.matmul(psh[:], sswT_sb[:], m3[:, r:r + 3, 0:128],
                                 start=True, stop=True)
                nc.scalar.activation(actv[:, r:r + 3, 1:129], psh[:], AF.Relu,
                                     bias=ssb_t, scale=1.0)

            # ---- PE: mu matmuls (fc Linear, all regions, one group) -----
            for j in range(F):
                for lc in range(4):
                    nc.tensor.matmul(mu_ps[:], cT[:, lc, j, :],
                                     fw_sb[:, j, lc, :],
                                     start=False, stop=(j == F - 1 and lc == 3))

            # ---- mu epilogue: relu, transpose ---------------------------
            mur = cst.tile([F, L], BF16)
            nc.scalar.activation(mur[:], mu_ps[:], AF.Relu)
            muT = cst.tile([128, 4, F], BF16)
            for kb in range(4):
                mt = paux.tile([128, F], BF16, tag="aux")
                nc.tensor.transpose(mt[:], mur[:, kb * 128:(kb + 1) * 128],
                                    id_bf[0:F, 0:F])
                nc.scalar.activation(muT[:, kb, :], mt[:], AF.Copy)

            # ---- PE: G matmuls, selG assembly spread over 3 queues ------
            gps = [gpsp.tile([F, 3, 128], F32, tag="gps", name=f"gps{_g}")
                   for _g in range(3)]
            for kb in range(4):
                for g in range(3):
                    nc.tensor.matmul(gps[g][:], muT[:, kb, :],
                                     wct_sb[:, kb, 3 * g:3 * g + 3, :],
                                     start=(kb == 0), stop=(kb == 3))
            selG = cst.tile([45, 128], BF16)
            gstage = cst.tile([F, 9, 128], BF16)
            for g in range(3):
                nc.scalar.activation(gstage[:, 3 * g:3 * g + 3, :],
                                     gps[g][:], AF.Copy)
            qs = [nc.sync, nc.gpsimd, nc.scalar]
            for t in range(9):
                qs[t % 3].dma_start(out=selG[F * t:F * t + F, :],
                                    in_=gstage[:, t, :])

            # ---- spade gamma/beta lhsT: DVE scale + 9 PE transposes -----
            nc.vector.tensor_scalar_mul(sgb[:], sgb[:], om_gba[:])
            spT = cst.tile([128, 9, 128], BF16)
            sgb3 = sgb[:].rearrange("p (l t) -> p l t", t=9)
            for t in range(9):
                pt = gpsp.tile([128, 128], BF16, tag="gps")
                nc.tensor.transpose(pt[:], sgb3[:, :, t], id_bf[:])
                nc.scalar.activation(spT[:, t, :], pt[:], AF.Copy)

            # ---- DVE: sel = seg * t, blend, then instance-norm stats ----
            stats_t = cst.tile([128, 16, 6], F32)
            x16 = xb_sb[:].rearrange("c (k n) -> c k n", k=16)
            for off, n in segchunks:
                nc.vector.tensor_mul(sel45[:, off:off + n],
                                     sel45[:, off:off + n],
                                     t_sb[:, off:off + n])
            nc.vector.tensor_scalar_mul(actv[:, 0, :], actv[:, 0, :],
                                        cpk[:, 3:4])
            nc.vector.tensor_scalar_mul(actv[:, SR - 1, :], actv[:, SR - 1, :],
                                        cpk[:, 4:5])
            for k in range(8):
                nc.vector.bn_stats(out=stats_t[:, k, :], in_=x16[:, k, :])
            # blend gamma/beta halves of selG while gstage lands
            nc.vector.tensor_mul(selG[:], selG[:], blendT[0:45, :])
            for k in range(8, 16):
                nc.vector.bn_stats(out=stats_t[:, k, :], in_=x16[:, k, :])

            # ---- instance-norm stats merge (packed half planes) ---------
            mv = cst.tile([128, 2], F32)
            nc.vector.bn_aggr(out=mv[:], in_=stats_t[:])
            mvb = cst.tile([C, 2], F32)
            nc.sync.dma_start(out=mvb[:], in_=mv[64:128, :])
            msum = cst.tile([C, 1], F32)
            nc.vector.tensor_add(msum[:], mv[0:64, 0:1], mvb[:, 0:1])
            mdif = cst.tile([C, 1], F32)
            nc.vector.tensor_sub(mdif[:], mv[0:64, 0:1], mvb[:, 0:1])
            vsum = cst.tile([C, 1], F32)
            nc.vector.tensor_add(vsum[:], mv[0:64, 1:2], mvb[:, 1:2])
            dm2 = cst.tile([C, 1], F32)
            nc.vector.tensor_mul(dm2[:], mdif[:], mdif[:])
            var_t = cst.tile([C, 1], F32)
            nc.vector.tensor_scalar(var_t[:], dm2[:], 0.25, None, op0=ALU.mult)
            nc.vector.tensor_scalar(vsum[:], vsum[:], 0.5, None, op0=ALU.mult)
            nc.vector.tensor_add(var_t[:], var_t[:], vsum[:])
            sd = cst.tile([C, 1], F32)
            nc.scalar.activation(sd[:], var_t[:], AF.Sqrt, bias=eps_t[:],
                                 scale=1.0)
            rstd = cst.tile([C, 1], F32)
            nc.vector.reciprocal(rstd[:], sd[:])
            nbias = cst.tile([C, 1], F32)
            nc.vector.tensor_scalar(nbias[:], msum[:], 0.5, None, op0=ALU.mult)
            nc.vector.tensor_mul(nbias[:], nbias[:], rstd[:])
            nc.vector.tensor_scalar_mul(nbias[:], nbias[:], -1.0)

            # ---- main conv + epilogue (epilogue one chunk behind) -------
            s3 = sel45[:].rearrange("p (r c) -> p r c", c=GW)
            pms = {}

            def conv_chunk(i):
                pm = pmain.tile([128, 4, 128], F32, tag="pm", name=f"pm_{i}")
                pms[i] = pm
                for t in range(9):
                    ty, tx = divmod(t, 3)
                    nc.tensor.matmul(pm[:], spT[:, t, :],
                                     actv[:, 4 * i + ty:4 * i + ty + 4,
                                          tx:tx + 128],
                                     start=(t == 0), stop=False)
                nc.tensor.matmul(pm[:], selG[:], s3[:, 4 * i:4 * i + 4, 0:128],
                                 start=False, stop=True)

            def epi_chunk(i):
                pm = pms.pop(i)
                gb = gbp.tile([128, 4, 128], BF16, tag="gb", name=f"gb_{i}")
                nc.scalar.activation(gb[:], pm[:], AF.Identity,
                                     bias=bias1_t[:], scale=1.0)
                pb = pbtp.tile([64, 4, 128], BF16, tag="pbt", name=f"pb_{i}")
                eng = nc.scalar if i % 2 == 0 else nc.gpsimd
                eng.dma_start(out=pb[:], in_=gb[64:128, :, :])
                xsl = xb_sb[0:64, i * 512:(i + 1) * 512]
                xnt = xnp.tile([C, 4, 128], F32, tag="xn", name=f"xnt_{i}")
                nc.gpsimd.tensor_scalar(
                    xnt[:].rearrange("p t c -> p (t c)"), xsl,
                    rstd[:], nbias[:], op0=ALU.mult, op1=ALU.add)
                nc.vector.tensor_mul(xnt[:], xnt[:], gb[0:64, :, :])
                nc.vector.tensor_add(xnt[:].rearrange("p t c -> p (t c)"),
                                     xnt[:].rearrange("p t c -> p (t c)"),
                                     pb[:].rearrange("p t c -> p (t c)"))
                if DBG and i == 3:
                    nc.sync.dma_start(out=dbg_gb[:],
                                      in_=gb[:].rearrange("c r w -> c (r w)"))
                nc.sync.dma_start(out=out_d[:, i, :],
                                  in_=xnt[:].rearrange("c r w -> c (r w)"))

            gb_keep = {}
            conv_chunk(0)
            for i in range(1, NCH):
                conv_chunk(i)
                epi_chunk(i - 1)
            epi_chunk(NCH - 1)
            if DBG:
                nc.sync.dma_start(out=dbg_mur[:], in_=mur[:])
                nc.sync.dma_start(out=dbg_selG[:], in_=selG[:])
                nc.sync.dma_start(out=dbg_actv[:],
                                  in_=actv[:, 1:4, :].rearrange("p a b -> p (a b)"))
                nc.sync.dma_start(out=dbg_spT[:],
                                  in_=spT[:, 0:2, :].rearrange("p a b -> p (a b)"))
                nc.sync.dma_start(out=dbg_sel45[:], in_=sel45[:, 2048:2560])
                nc.sync.dma_start(out=dbg_muTg[:],
                                  in_=muT[:].rearrange("p a b -> p (a b)"))
                nc.sync.dma_start(out=dbg_stats[:, 0:1], in_=rstd[:])
                nc.sync.dma_start(out=dbg_stats[:, 1:2], in_=nbias[:])
                nc.sync.dma_start(out=dbg_wct[:],
                                  in_=wct_sb[:, 0, :, :].rearrange("p a b -> p (a b)"))
                nc.sync.dma_start(out=dbg_gst[:],
                                  in_=gstage[:].rearrange("p a b -> p (a b)"))

    nc.finalize()
    return nc


_NC = None


def _cpk(cgb, cbb, sgbb, sbbb, ssb, bg, bb, h0):
    cpk = np.zeros((128, 8), np.float32)
    cpk[0:64, 0] = cgb; cpk[64:128, 0] = cbb
    cpk[0:64, 1] = sgbb; cpk[64:128, 1] = sbbb
    cpk[:, 2] = ssb
    cpk[:, 3] = 0.0 if h0 == 0 else 1.0
    cpk[:, 4] = 0.0 if h0 + ROWS == H else 1.0
    cpk[:, 5] = bg[0]
    cpk[:, 6] = bb[0]
    return cpk


def kernel(**inputs):
    global _NC
    x = np.asarray(inputs["x"], dtype=np.float32)
    segmap = np.asarray(inputs["segmap"], dtype=np.float32)
    codes_vector = np.asarray(inputs["codes_vector"], dtype=np.float32)
    mask = np.asarray(inputs["mask"], dtype=np.float32)
    fc_w = np.asarray(inputs["fc_w"], dtype=np.float32)
    fc_b = np.asarray(inputs["fc_b"], dtype=np.float32)
    conv_gamma_w = np.asarray(inputs["conv_gamma_w"], dtype=np.float32)
    conv_gamma_b = np.asarray(inputs["conv_gamma_b"], dtype=np.float32)
    conv_beta_w = np.asarray(inputs["conv_beta_w"], dtype=np.float32)
    conv_beta_b = np.asarray(inputs["conv_beta_b"], dtype=np.float32)
    spade_shared_w = np.asarray(inputs["spade_shared_w"], dtype=np.float32)
    spade_shared_b = np.asarray(inputs["spade_shared_b"], dtype=np.float32)
    spade_gamma_w = np.asarray(inputs["spade_gamma_w"], dtype=np.float32)
    spade_gamma_b = np.asarray(inputs["spade_gamma_b"], dtype=np.float32)
    spade_beta_w = np.asarray(inputs["spade_beta_w"], dtype=np.float32)
    spade_beta_b = np.asarray(inputs["spade_beta_b"], dtype=np.float32)
    blending_gamma = np.asarray(inputs["blending_gamma"], dtype=np.float32)
    blending_beta = np.asarray(inputs["blending_beta"], dtype=np.float32)

    if _NC is None:
        _NC = _build_nc()

    # fc weights: mu[j,k] = sum_l codes[j,l] * fc_w[j,k,l] -> rhs tiles
    # [128(l_part), 512(k)] per (j, lc): fcwT[j][p, lc*512+k] = fc_w[j,k,lc*128+p]
    fcwT_h = np.ascontiguousarray(
        fc_w.transpose(0, 2, 1).reshape(F, 4, 128, 512).transpose(2, 0, 1, 3)
        .reshape(128, F * 4 * 512)).astype(NPBF)
    # region conv taps: wct[p, lc, t, c] = Wconv[c, lc*128+p, t]
    wc = np.concatenate([conv_gamma_w, conv_beta_w], axis=0)  # [128c, 512, 3, 3]
    wctd_h = np.ascontiguousarray(
        wc.reshape(128, 512, 9).transpose(1, 2, 0)      # [512l, 9t, 128c]
        .reshape(4, 128, 9, 128).transpose(1, 0, 2, 3)  # [128p, 4lc, 9, 128]
        .reshape(128, 4 * 9 * 128)).astype(NPBF)
    # spade gamma/beta stacked, natural layout (device transposes these 9)
    sgbd_h = np.concatenate(
        [spade_gamma_w.reshape(C, NH * 9), spade_beta_w.reshape(C, NH * 9)],
        axis=0).astype(NPBF)
    # shared conv lhsT [27(ty,tx,ic), NH]
    sswT_h = np.ascontiguousarray(
        spade_shared_w.transpose(2, 3, 1, 0).reshape(27, NH)).astype(NPBF)

    shared = {
        "fcwT": fcwT_h,
        "wctd": wctd_h,
        "sgbd": sgbd_h,
        "sswT": sswT_h,
        "fcb": np.ascontiguousarray(fc_b).astype(NPBF),
        "u5": np.kron(np.eye(9, dtype=np.float32),
                      np.tril(np.ones((F, F), np.float32), -1)).astype(NPBF),
        "ident": np.eye(128, dtype=np.float32).astype(NPBF),
    }

    in_maps = []
    for c in range(NCORES):
        b, half = divmod(c, 2)
        h0 = half * ROWS
        segp = np.zeros((F, SEG_SZ), np.float32)
        segp2 = np.zeros((F, SR, GW), np.float32)
        r_lo, r_hi = h0 - 1, h0 + ROWS + 1  # exclusive
        s_lo, s_hi = max(r_lo, 0), min(r_hi, H)
        segp2[:, s_lo - r_lo:s_hi - r_lo, 1:129] = segmap[b, :, s_lo:s_hi, :]
        segp[:, 0:SR * GW] = segp2.reshape(F, -1)
        maskp = np.zeros((3, MASK_SZ), np.float32)
        maskp2 = np.zeros((3, MR, GW), np.float32)
        m_lo, m_hi = h0 - 2, h0 + ROWS + 2
        ms_lo, ms_hi = max(m_lo, 0), min(m_hi, H)
        maskp2[:, ms_lo - m_lo:ms_hi - m_lo, 1:129] = mask[b, :, ms_lo:ms_hi, :]
        maskp[:, 0:MR * GW] = maskp2.reshape(3, -1)
        # x plane rotated so the core's own 64 rows come first: the epilogue
        # reads columns [i*512, (i+1)*512) directly; stats are rotation-
        # invariant.
        xrot = np.roll(x[b].reshape(C, H, W), -h0, axis=1).reshape(C, H * W)
        xpk = np.concatenate([xrot[:, :H * W // 2], xrot[:, H * W // 2:]], 0)
        # block-diagonal codes lhsT [128(p), 4(lc), F(j), F(col)]: column j of
        # the (lc, j) slice holds codes_vector[b, j, lc*128+p], rest zero.
        cT_full = codes_vector[b].T.reshape(4, 128, F).transpose(1, 0, 2)
        codes5 = np.zeros((128, 4, F, F), np.float32)
        for j in range(F):
            codes5[:, :, j, j] = cT_full[:, :, j]
        codes2_h = np.ascontiguousarray(
            codes5.reshape(128, 4 * F * F)).astype(NPBF)
        in_maps.append(dict(
            shared,
            xb=xpk.astype(NPBF),
            cpkd=_cpk(conv_gamma_b, conv_beta_b, spade_gamma_b, spade_beta_b,
                      spade_shared_b, blending_gamma, blending_beta, h0),
            segg=np.ascontiguousarray(segp).astype(NPBF),
            maskg=np.ascontiguousarray(maskp).astype(NPBF),
            codes2=codes2_h,
        ))

    res = run_bass_kernel_spmd(_NC, in_maps, list(range(NCORES)))

    out = np.empty((B, C, H, W), np.float32)
    for c in range(NCORES):
        b, half = divmod(c, 2)
        h0 = half * ROWS
        out[b, :, h0:h0 + ROWS, :] = res.results[c]["out"].reshape(C, ROWS, W)
    return out
